# revision 7
# baseline (speedup 1.0000x reference)
"""GQA attention forward (dense_transformer), 8-core tensor-parallel Bass kernel.

v2: software-pipelined emission. Two interleaved instruction streams keep the
PE busy end-to-end: S_mm (kv/q projections, dense) and S_at (attention
scores/exp/PV + AllGather chunks). Key changes vs v1:
 - hs loaded via 4 parallel DMA queues, chunked by a-tile, so the first
   projection matmul starts ~10us in instead of ~76us.
 - rotate-half RoPE via partition-offset DVE multiplies (no SBUF->SBUF
   shuffle DMAs, which cost 14us and stalled attention).
 - 5 chunked AllGathers (per 512/256-token block) instead of 2 whole-batch
   ones; dense runs per-chunk as gathers land, so the tail is one small
   AG + one 256-wide dense chunk.
 - softmax row-sum reciprocal via reciprocal_approx_fast (5x faster).
 - PSUM: acc(2) + st(2) + pv(3) + vtp(1) = 8 banks exactly.
Problem (hardcoded): B=2, S=1024, H=4096, n_kv=8, G=8, D=64, f32 io.
Core m owns kv-group m (8 q-heads + 1 kv-head); output columns
y[:, m*512:(m+1)*512] computed from AllGathered attention features,
emitted transposed; host un-transposes and concatenates.
"""

import sys

import numpy as np

for _p in ("/opt/trn_rl_repo",):
    if _p not in sys.path:
        sys.path.insert(0, _p)

import ml_dtypes

B, S, H = 2, 1024, 4096
NKV, G, D = 8, 8, 64
NC = 8
BS = B * S          # 2048 flattened tokens
EL = G * D          # 512 local attn features per core
HT = H // 128       # 32 h-tiles
SBK = 512
INV = 0.125         # 1/sqrt(D)

# attention/AG blocks per batch: (query_start_within_batch, width)
BLOCKS = {0: [(0, 512), (512, 512)], 1: [(0, 512), (512, 256), (768, 256)]}

_CACHE = {}


def _fix_bir_for_old_walrus(bir_json):
    """Adapt newer-concourse BIR to the container's older walrus:
    - register allocations need num_physical_regs set
    - only one sem-wait per instruction: hoist extras onto EventSemaphore nops
    - drop Ldweights identical to the previous one (stationary persists in
      the PE array; verified bit-exact on hardware)
    """
    import json

    bir = json.loads(bir_json)
    for f in bir["functions"]:
        for a in f["allocations"]:
            if a.get("Skind") == "register" and not a.get("num_physical_regs"):
                a["num_physical_regs"] = 1
        for blk in f["blocks"]:
            newins = []
            last_ldw = None
            for ins in blk["instructions"]:
                si = ins.get("sync_info") or {}
                waits = si.get("on_wait") or []
                if len(waits) > 1:
                    for j, w in enumerate(waits[:-1]):
                        newins.append(
                            {
                                "engine": ins["engine"],
                                "ins": [],
                                "outs": [],
                                "name": f"{ins['name']}_w{j}",
                                "opcode": "EventSemaphore",
                                "sync_info": {"on_update": [], "on_wait": [w]},
                                "debug": ins.get("debug"),
                            }
                        )
                    si["on_wait"] = [waits[-1]]
                op = ins.get("opcode")
                if op == "Ldweights":
                    key = json.dumps(ins["ins"], sort_keys=True)
                    sync = ins.get("sync_info") or {}
                    if (
                        key == last_ldw
                        and not sync.get("on_wait")
                        and not sync.get("on_update")
                    ):
                        continue
                    last_ldw = key
                elif op != "Matmult":
                    last_ldw = None
                newins.append(ins)
            blk["instructions"] = newins
    return json.dumps(bir).encode()


def _install_compiler_shim():
    if _CACHE.get("shim"):
        return
    import concourse.bass_utils as bu
    import concourse.bass2jax as b2j

    orig = getattr(bu.compile_bir_kernel, "__wrapped__", bu.compile_bir_kernel)

    def patched(bir_json, tmpdir, neff_name="file.neff"):
        return orig(_fix_bir_for_old_walrus(bir_json), tmpdir, neff_name)

    bu.compile_bir_kernel = patched
    b2j.compile_bir_kernel = patched
    _CACHE["shim"] = True


def build(debug=False):
    _install_compiler_shim()
    import concourse.bass as bass  # noqa: F401
    import concourse.mybir as mybir
    import concourse.tile as tile
    from concourse import bacc

    fp32 = mybir.dt.float32
    bf16 = mybir.dt.bfloat16
    AF = mybir.ActivationFunctionType
    ALU = mybir.AluOpType

    nc = bacc.Bacc("TRN2", debug=debug, target_bir_lowering=False, num_devices=NC)

    hs3 = nc.declare_dram_parameter("hs3", [HT, 128, BS], bf16, isOutput=False)
    wqT = nc.declare_dram_parameter("wqT", [H, EL], bf16, isOutput=False)
    wkvT = nc.declare_dram_parameter("wkvT", [H, 2 * D], bf16, isOutput=False)
    wdT = nc.declare_dram_parameter("wdT", [H, EL], bf16, isOutput=False)
    cosq = nc.declare_dram_parameter("cosq", [128, BS], bf16, isOutput=False)
    sinq = nc.declare_dram_parameter("sinq", [128, BS], bf16, isOutput=False)
    trimask = nc.declare_dram_parameter("trimask", [128, 128], bf16, isOutput=False)
    ident = nc.declare_dram_parameter("ident", [128, 64], bf16, isOutput=False)
    outT = nc.declare_dram_parameter("outT", [EL, BS], fp32, isOutput=True)

    rg = [list(range(NC))]

    with tile.TileContext(nc, num_cores=NC) as tc:
        with (
            tc.tile_pool(name="const", bufs=1) as cp,
            tc.tile_pool(name="dram", bufs=1, space="DRAM") as dp,
            tc.tile_pool(name="mid", bufs=1) as mp,
            tc.tile_pool(name="tmp", bufs=1) as tp,
            tc.tile_pool(name="acc", bufs=2, space="PSUM") as pacc,
            tc.tile_pool(name="stp", bufs=2, space="PSUM") as pst,
            tc.tile_pool(name="pvp", bufs=3, space="PSUM") as ppv,
            tc.tile_pool(name="vtp", bufs=1, space="PSUM") as pvt,
        ):
            wq_sb = cp.tile([128, HT, EL], bf16)
            wkv_sb = cp.tile([128, HT, 2 * D], bf16)
            wd_sb = cp.tile([128, HT, EL], bf16)
            cos_sb = cp.tile([128, BS], bf16)
            sin_sb = cp.tile([128, BS], bf16)
            tri_sb = cp.tile([128, 128], bf16)
            id_sb = cp.tile([128, 64], bf16)

            qT_sb = mp.tile([128, 4, BS], bf16)
            kT_sb = mp.tile([128, BS], bf16)
            v_ext = mp.tile([128, BS // 128, D + 1], bf16)

            hs_r = hs3.ap().rearrange("a p s -> p a s")
            wq_r = wqT.ap().rearrange("(a p) e -> p a e", p=128)
            wd_r = wdT.ap().rearrange("(a p) e -> p a e", p=128)

            # DRAM staging for the chunked AllGathers
            agin = {}
            agout = {}
            for b in (0, 1):
                for (q0, w) in BLOCKS[b]:
                    agin[(b, q0)] = dp.tile([EL, w], bf16, name=f"agin{b}_{q0}")
                    agout[(b, q0)] = dp.tile(
                        [NC * EL, w], bf16, addr_space="Shared",
                        name=f"agout{b}_{q0}",
                    )

            # ---- prologue: constants + batch-0 hs across 4 DMA queues
            nc.gpsimd.dma_start(
                wkv_sb[:], wkvT.ap().rearrange("(a p) e -> p a e", p=128)
            )
            hs_t = {0: mp.tile([128, HT, S], bf16, tag="big", name="hs0")}
            h_engines = [nc.sync, nc.scalar] * 4
            for c in range(8):
                h_engines[c].dma_start(
                    hs_t[0][:, c * 4 : (c + 1) * 4, :],
                    hs_r[:, c * 4 : (c + 1) * 4, 0:S],
                )
            for c in range(4):
                nc.gpsimd.dma_start(
                    wq_sb[:, c * 8 : (c + 1) * 8, :], wq_r[:, c * 8 : (c + 1) * 8, :]
                )
            nc.gpsimd.dma_start(cos_sb[:], cosq.ap())
            nc.gpsimd.dma_start(sin_sb[:], sinq.ap())
            nc.gpsimd.dma_start(tri_sb[:], trimask.ap())
            nc.gpsimd.dma_start(id_sb[:], ident.ap())
            nc.gpsimd.memset(v_ext[:, :, D : D + 1], 1.0)

            kvraws = {}   # (b, sh) -> kvraw tile, for deferred v transposes
            agc = {}      # b -> gathered-activation SBUF tile

            def emit_vt(b, j):
                # one V-transpose: [d,t]->[t,d] via PE, into v_ext tile b*8+j
                sh, jj = divmod(j, 4)
                vtp = pvt.tile([128, 64], bf16, tag="vt")
                nc.tensor.transpose(
                    vtp[:],
                    kvraws[(b, sh)][64:128, jj * 128 : (jj + 1) * 128],
                    id_sb[64:128, :],
                )
                nc.scalar.copy(v_ext[:, b * 8 + sh * 4 + jj, 0:D], vtp[:])

            def kv_group(b):
                for sh in range(2):
                    sc = slice(b * S + sh * 512, b * S + (sh + 1) * 512)
                    ac = pacc.tile([128, SBK], fp32, tag="acc")
                    for a in range(HT):
                        nc.tensor.matmul(
                            ac[:],
                            lhsT=wkv_sb[:, a, :],
                            rhs=hs_t[b][:, a, sh * 512 : (sh + 1) * 512],
                            start=(a == 0),
                            stop=(a == HT - 1),
                        )
                        if a % 8 == 7 and a < HT - 1:
                            yield None
                    kvraw = tp.tile([128, SBK], bf16, tag="kvraw", bufs=2)
                    kvraws[(b, sh)] = kvraw
                    nc.vector.tensor_copy(kvraw[:], ac[:])
                    # rotate_half via partition-shifted single-input copies
                    # (TensorTensor requires equal input base partitions)
                    ksh = tp.tile([64, SBK], bf16, tag="ksh", bufs=1)
                    nc.vector.tensor_copy(ksh[0:32, :], ac[32:64, :])
                    nc.gpsimd.tensor_copy(ksh[32:64, :], kvraw[0:32, :])
                    kt1 = tp.tile([64, SBK], bf16, tag="kt1", bufs=1)
                    kt2 = tp.tile([64, SBK], bf16, tag="kt2", bufs=1)
                    nc.vector.tensor_mul(kt1[:], kvraw[0:64, :], cos_sb[0:64, sc])
                    nc.vector.tensor_mul(kt2[:], ksh[:], sin_sb[0:64, sc])
                    nc.vector.tensor_add(kT_sb[0:64, sc], kt1[:], kt2[:])
                    # duplicate into upper half so odd heads' scores can use it
                    nc.gpsimd.tensor_copy(kT_sb[64:128, sc], kT_sb[0:64, sc])
                    yield None

            def q_group(b, et):
                for sh in range(2):
                    sc = slice(b * S + sh * 512, b * S + (sh + 1) * 512)
                    ac = pacc.tile([128, SBK], fp32, tag="acc")
                    for a in range(HT):
                        nc.tensor.matmul(
                            ac[:],
                            lhsT=wq_sb[:, a, et * 128 : (et + 1) * 128],
                            rhs=hs_t[b][:, a, sh * 512 : (sh + 1) * 512],
                            start=(a == 0),
                            stop=(a == HT - 1),
                        )
                        if a % 8 == 7:
                            if et == 0:
                                emit_vt(b, sh * 4 + (a // 8))
                            if a < HT - 1:
                                yield None
                    qraw = tp.tile([128, SBK], bf16, tag="qraw", bufs=2)
                    nc.vector.tensor_copy(qraw[:], ac[:])
                    qsh = tp.tile([128, SBK], bf16, tag="qsh", bufs=2)
                    for hh in range(2):
                        for half in range(2):
                            dst = slice(hh * 64 + half * 32, hh * 64 + half * 32 + 32)
                            src = slice(
                                hh * 64 + (1 - half) * 32,
                                hh * 64 + (1 - half) * 32 + 32,
                            )
                            if half == 0:
                                nc.vector.tensor_copy(qsh[dst, :], ac[src, :])
                            else:
                                nc.gpsimd.tensor_copy(qsh[dst, :], qraw[src, :])
                    t1 = tp.tile([128, SBK], bf16, tag="t1", bufs=2)
                    t2 = tp.tile([128, SBK], bf16, tag="t2", bufs=2)
                    nc.vector.tensor_mul(t1[:], qraw[:], cos_sb[:, sc])
                    nc.vector.tensor_mul(t2[:], qsh[:], sin_sb[:, sc])
                    nc.vector.tensor_add(qT_sb[:, et, sc], t1[:], t2[:])
                    yield None

            def attn_block(b, pair, q0, w):
                ge, go = 2 * pair, 2 * pair + 1
                sc = slice(b * S + q0, b * S + q0 + w)
                ntile = (q0 + w) // 128
                t0 = q0 // 128
                pvs = [
                    ppv.tile([D + 1, SBK], fp32, tag="pv", name=f"pv{b}_{pair}_{q0}_{gi}")
                    for gi in range(2)
                ]

                def emit_pv(ti, pts):
                    for gi in range(2):
                        nc.tensor.matmul(
                            pvs[gi][:, 0:w],
                            lhsT=v_ext[:, b * 8 + ti, :],
                            rhs=pts[gi][:, 0:w],
                            start=(ti == 0),
                            stop=(ti == ntile - 1),
                        )

                prev = None
                for ti in range(ntile):
                    tcol = slice(b * S + ti * 128, b * S + (ti + 1) * 128)
                    k = ti - t0
                    sts = []
                    for g in (ge, go):
                        rows = slice((g % 2) * 64, (g % 2) * 64 + 64)
                        st = pst.tile([128, SBK], fp32, tag="st")
                        nc.tensor.matmul(
                            st[:, 0:w],
                            lhsT=kT_sb[rows, tcol],
                            rhs=qT_sb[rows, pair, sc],
                            start=True,
                            stop=True,
                        )
                        sts.append(st)
                    pts = []
                    for gi in range(2):
                        pT = tp.tile([128, SBK], bf16, tag="pt", bufs=8)
                        if k < 0:
                            nc.scalar.activation(
                                pT[:, 0:w], sts[gi][:, 0:w], AF.Exp, scale=INV
                            )
                        else:
                            if k > 0:
                                nc.gpsimd.memset(pT[:, 0 : k * 128], 0.0)
                            nc.scalar.activation(
                                pT[:, k * 128 : w], sts[gi][:, k * 128 : w],
                                AF.Exp, scale=INV,
                            )
                            nc.vector.tensor_mul(
                                pT[:, k * 128 : (k + 1) * 128],
                                pT[:, k * 128 : (k + 1) * 128],
                                tri_sb[:],
                            )
                        pts.append(pT)
                    if prev is not None:
                        emit_pv(*prev)
                    prev = (ti, pts)
                    yield None
                emit_pv(*prev)
                # normalize + ship to DRAM for the AllGather (baseline mechanics)
                for gi, g in enumerate((ge, go)):
                    pv = pvs[gi]
                    rc = tp.tile([1, SBK], bf16, tag="rc", bufs=2)
                    with nc.allow_low_precision(reason="softmax recip bf16"):
                        nc.vector.reciprocal(rc[0:1, 0:w], pv[D : D + 1, 0:w])
                    bcs = tp.tile([64, SBK], bf16, tag="bcs", bufs=2)
                    nc.gpsimd.partition_broadcast(bcs[:, 0:w], rc[0:1, 0:w])
                    ao = tp.tile([64, SBK], bf16, tag="ao", bufs=2)
                    nc.vector.tensor_mul(ao[:, 0:w], pv[0:D, 0:w], bcs[:, 0:w])
                    nc.sync.dma_start(
                        agin[(b, q0)][g * 64 : (g + 1) * 64, 0:w], ao[:, 0:w]
                    )
                yield None

            def dense_chunk(b, q0, w, require):
                yield ("require", require)
                if b not in agc:
                    agc[b] = mp.tile([128, HT, S], bf16, tag="big", name=f"agc{b}")
                agr = agout[(b, q0)].rearrange("(a p) s -> p a s", p=128)
                for c in range(8):
                    eng = (nc.sync, nc.gpsimd)[c % 2]
                    eng.dma_start(
                        agc[b][:, c * 4 : (c + 1) * 4, q0 : q0 + w],
                        agr[:, c * 4 : (c + 1) * 4, :],
                    )
                for ot in range(4):
                    ac = pacc.tile([128, SBK], fp32, tag="acc")
                    for a in range(HT):
                        nc.tensor.matmul(
                            ac[:, 0:w],
                            lhsT=wd_sb[:, a, ot * 128 : (ot + 1) * 128],
                            rhs=agc[b][:, a, q0 : q0 + w],
                            start=(a == 0),
                            stop=(a == HT - 1),
                        )
                        if a % 8 == 7 and a < HT - 1:
                            yield None
                    ysb = tp.tile([128, SBK], fp32, tag="ysb", bufs=2)
                    nc.vector.tensor_copy(ysb[:, 0:w], ac[:, 0:w])
                    nc.sync.dma_start(
                        outT.ap()[ot * 128 : (ot + 1) * 128, b * S + q0 : b * S + q0 + w],
                        ysb[:, 0:w],
                    )
                    yield None

            def s_mm():
                for b in (0, 1):
                    if b == 1:
                        hs_t[1] = mp.tile([128, HT, S], bf16, tag="big", name="hs1")
                        b1_engines = [nc.scalar, nc.sync] * 4
                        for c in range(8):
                            b1_engines[c].dma_start(
                                hs_t[1][:, c * 4 : (c + 1) * 4, :],
                                hs_r[:, c * 4 : (c + 1) * 4, S : 2 * S],
                            )
                    yield from kv_group(b)
                    for et in range(4):
                        yield from q_group(b, et)
                        yield ("milestone", f"q{b}_{et}")
                        if b == 1:
                            nc.gpsimd.dma_start(
                                wd_sb[:, et * 8 : (et + 1) * 8, :],
                                wd_r[:, et * 8 : (et + 1) * 8, :],
                            )
                # lag-one AG gating: dense for chunk i is emitted only after the
                # NEXT chunk's AllGather is emitted, so attention work queues
                # ahead of AG-blocked dense matmuls (no PE head-of-line stall).
                chunks = [(b, q0, w) for b in (0, 1) for (q0, w) in BLOCKS[b]]
                reqs = [f"ag{b}_{q0}" for (b, q0, w) in chunks]
                lagged = reqs[1:] + [reqs[-1]]
                for (b, q0, w), req in zip(chunks, lagged):
                    yield from dense_chunk(b, q0, w, req)

            def s_at():
                for b in (0, 1):
                    for bi, (q0, w) in enumerate(BLOCKS[b]):
                        for pair in range(4):
                            need = pair if bi == 0 else 3
                            yield ("require", f"q{b}_{need}")
                            yield from attn_block(b, pair, q0, w)
                        nc.gpsimd.collective_compute(
                            "AllGather",
                            ALU.bypass,
                            replica_groups=rg,
                            ins=[agin[(b, q0)][:].opt()],
                            outs=[agout[(b, q0)][:].opt()],
                        )
                        yield ("milestone", f"ag{b}_{q0}")

            # ---- driver: alternate quanta between the two streams.
            # S_at gets 2 quanta per round: attention (and its AllGathers)
            # finish early so the collectives overlap the dense tail.
            milestones = set()
            blocked = {}
            g_mm, g_at = s_mm(), s_at()
            weights = {g_mm: 1, g_at: 2}
            active = [g_mm, g_at]
            while active:
                progressed = False
                for g in list(active):
                    if g in blocked:
                        if blocked[g] in milestones:
                            del blocked[g]
                        else:
                            continue
                    try:
                        for _ in range(weights[g]):
                            while True:
                                ev = next(g)
                                if ev is None:
                                    break
                                kind, name = ev
                                if kind == "milestone":
                                    milestones.add(name)
                                elif kind == "require" and name not in milestones:
                                    blocked[g] = name
                                    break
                            if g in blocked:
                                break
                    except StopIteration:
                        active.remove(g)
                    progressed = True
                if not progressed:
                    raise RuntimeError(
                        f"emission deadlock: blocked={blocked} ms={milestones}"
                    )

    nc.finalize()
    return nc


def _prep_inputs(hidden_states, cos, sin, wq, wk, wv, wd):
    bf = ml_dtypes.bfloat16
    hs3 = np.ascontiguousarray(
        hidden_states.reshape(BS, H).T.reshape(HT, 128, BS)
    ).astype(bf)
    cosT = cos.T.astype(np.float32)  # [64, 1024]
    sinT = sin.T.astype(np.float32)
    sinS = np.concatenate([-sinT[0:32], sinT[32:64]], axis=0)
    cosq = np.ascontiguousarray(np.tile(cosT, (2, 2))).astype(bf)  # [128, 2048]
    sinq = np.ascontiguousarray(np.tile(sinS, (2, 2))).astype(bf)
    tri = np.triu(np.ones((128, 128), dtype=np.float32)).astype(bf)
    idn = np.zeros((128, 64), dtype=np.float32)
    idn[0:64] = np.eye(64)
    idn[64:128] = np.eye(64)
    idn = idn.astype(bf)
    in_maps = []
    for m in range(NC):
        wkv = np.concatenate(
            [wk[m * D : (m + 1) * D, :], wv[m * D : (m + 1) * D, :]], axis=0
        )  # [128, H]
        in_maps.append(
            {
                "hs3": hs3,
                "wqT": np.ascontiguousarray(wq[m * EL : (m + 1) * EL, :].T.astype(bf)),
                "wkvT": np.ascontiguousarray(wkv.T.astype(bf)),
                "wdT": np.ascontiguousarray(wd[m * EL : (m + 1) * EL, :].T.astype(bf)),
                "cosq": cosq,
                "sinq": sinq,
                "trimask": tri,
                "ident": idn,
            }
        )
    return in_maps


def kernel(hidden_states, alibi, attention_mask, cos, sin, wq, wk, wv, wd,
           _trace=False):
    from concourse.bass_utils import run_bass_kernel_spmd

    if "nc" not in _CACHE:
        _CACHE["nc"] = build()
    nc = _CACHE["nc"]
    in_maps = _prep_inputs(hidden_states, cos, sin, wq, wk, wv, wd)
    res = run_bass_kernel_spmd(nc, in_maps, core_ids=list(range(NC)), trace=_trace)
    _CACHE["last_result"] = res
    outs = [
        np.ascontiguousarray(res.results[m]["outT"].T).reshape(B, S, EL)
        for m in range(NC)
    ]
    return np.concatenate(outs, axis=-1).astype(np.float32)
